# revision 1
# baseline (speedup 1.0000x reference)
"""Trainium2 Bass kernel for nn_LinearAttention (gated linear attention).

Math (per reference):
    qkv = x @ Wqkv.T ; q,k,v = split(qkv); q,k = elu(.)+1
    per (b,h): running_kv[t]  = d*running_kv[t-1]  + k[t]*v[t]   (elementwise, D=64)
               running_ksum[t]= d*running_ksum[t-1]+ k[t]
    den = clip(sum_d(q*running_ksum), 1e-6); out = q*running_kv/den
    g = sigmoid(out @ Wgate.T + bgate); out = g*out + (1-g)*v
    y = out @ Wout.T

Implementation strategy (8 NeuronCores, SPMD, no collectives):
  - Token-parallel: core c handles batch b=c//2, T-half h=c%2 (2048 tokens)
    plus a 512-token halo before the chunk to warm the decay scan
    (decay=0.95 => truncation error ~0.95^512 ~ 4e-12).  Half 0 gets a
    zero halo + k-mask so its scan state is exactly 0 at t=0.
  - Everything on-chip lives as [feature(partition), token(free)]; the host
    pre-transposes x and the weight matrices so both matmul operands are in
    natural layout and no on-chip transpose is ever needed.  The final
    output is produced transposed ([hidden, T]) and un-transposed on host.
  - The decay scan runs natively on the Vector engine via
    tensor_tensor_scan (state = d*state + u along the free/time axis),
    chained across 512-token groups via initial=prev[:, -1:].
  - den: sum over D=64 partitions via a 0/1 block-diagonal selector matmul
    (PSUM [16,512]); reciprocal broadcast back to 128 partitions via a
    second selector matmul in fp32r.
  - phi(x)=elu(x)+1 = exp(min(x,0)) + relu(x): DVE min, ACT Exp, then one
    fused scalar_tensor_tensor (max 0 then add).
  - bgate rides the Sigmoid drain as the ACT per-partition bias.
"""

import sys

for _p in ('/opt/trn_rl_repo', '/root/.axon_site'):
    if _p not in sys.path:
        sys.path.insert(0, _p)

from contextlib import ExitStack

import ml_dtypes
import numpy as np

import concourse.tile as tile
from concourse import bacc, mybir
from concourse.bass_utils import run_bass_kernel_spmd

F32 = mybir.dt.float32
BF16 = mybir.dt.bfloat16
AL = mybir.AluOpType
AF = mybir.ActivationFunctionType

B, T, HID = 4, 4096, 1024
H, D = 16, 64
OD = 3 * HID              # 3072 qkv output rows
NK = HID // 128           # 8 hidden (contraction) tiles
NOT = OD // 128           # 24 od tiles: q=0..7, k=8..15, v=16..23
HALF_T = T // 2           # 2048 tokens per core
HALO = 512
TLOC = HALO + HALF_T      # 2560
WG = 512                  # token-group width
NG = TLOC // WG           # 5 groups; group 0 = halo
NH = HID // 128           # 8 tiles per q/k/v section

_cache = {}


def _build_nc():
    nc = bacc.Bacc("TRN2", target_bir_lowering=False, debug=False)

    xT = nc.dram_tensor("xT", [HID, TLOC], BF16, kind="ExternalInput")
    wqkvT = nc.dram_tensor("wqkvT", [HID, OD], BF16, kind="ExternalInput")
    wgateT = nc.dram_tensor("wgateT", [HID, HID], BF16, kind="ExternalInput")
    woutT = nc.dram_tensor("woutT", [HID, HID], BF16, kind="ExternalInput")
    dec_c = nc.dram_tensor("dec_c", [128, NH], F32, kind="ExternalInput")
    mask_c = nc.dram_tensor("mask_c", [128, 1], F32, kind="ExternalInput")
    densel = nc.dram_tensor("densel", [128, NH * H], BF16, kind="ExternalInput")
    bcsel = nc.dram_tensor("bcsel", [H, NH * 128], mybir.dt.float32r,
                           kind="ExternalInput")
    bgate_c = nc.dram_tensor("bgate_c", [128, NH], F32, kind="ExternalInput")
    yT = nc.dram_tensor("yT", [HID, HALF_T], F32, kind="ExternalOutput")

    with tile.TileContext(nc) as tc, ExitStack() as ctx:
        consts = ctx.enter_context(tc.tile_pool(name="consts", bufs=1))
        wq_pool = ctx.enter_context(tc.tile_pool(name="wq", bufs=1))
        wg_pool = ctx.enter_context(tc.tile_pool(name="wgp", bufs=1))
        wo_pool = ctx.enter_context(tc.tile_pool(name="wop", bufs=1))
        xt_pool = ctx.enter_context(tc.tile_pool(name="xt", bufs=16))
        qkv_pool = ctx.enter_context(tc.tile_pool(name="qkv", bufs=9))
        tmp_pool = ctx.enter_context(tc.tile_pool(name="tmp", bufs=2))
        cum_pool = ctx.enter_context(tc.tile_pool(name="cum", bufs=1))
        st_pool = ctx.enter_context(tc.tile_pool(name="st", bufs=2))
        oa_pool = ctx.enter_context(tc.tile_pool(name="oa", bufs=9))
        gt_pool = ctx.enter_context(tc.tile_pool(name="gt", bufs=2))
        mix_pool = ctx.enter_context(tc.tile_pool(name="mix", bufs=9))
        y_pool = ctx.enter_context(tc.tile_pool(name="ysb", bufs=2))
        ps_pool = ctx.enter_context(tc.tile_pool(name="ps", bufs=7, space="PSUM"))
        psd_pool = ctx.enter_context(tc.tile_pool(name="psd", bufs=1, space="PSUM"))

        # small consts first (mask gates the halo k drains)
        dec_s = consts.tile([128, NH], F32, tag="dec")
        nc.gpsimd.dma_start(dec_s[:], dec_c.ap()[:, :])
        mask_s = consts.tile([128, 1], F32, tag="mask")
        nc.gpsimd.dma_start(mask_s[:], mask_c.ap()[:, :])

        # qkv weights: one SBUF tile per hid k-tile, loaded by od section in
        # the order the pipeline consumes them: k-sections, then (after the
        # first x tiles, emitted by the pipeline below) v- and q-sections.
        wq_sec = {}
        for sec in range(3):
            wq_sec[sec] = []
            for k in range(NK):
                w = wq_pool.tile([128, HID], BF16, tag=f"wq{sec}_{k}",
                                 name=f"wq_{sec}_{k}")
                wq_sec[sec].append(w)

        def load_wq_sec(sec, eng=None):
            eng = eng or nc.sync
            for k in range(NK):
                eng.dma_start(
                    wq_sec[sec][k][:],
                    wqkvT.ap()[128 * k:128 * (k + 1), HID * sec:HID * (sec + 1)])

        load_wq_sec(1)  # k-section: first thing the PE needs

        densel_s = consts.tile([128, NH * H], BF16, tag="densel")
        bcsel_s = consts.tile([H, NH * 128], mybir.dt.float32r, tag="bcsel")
        bgate_s = consts.tile([128, NH], F32, tag="bg")
        wg_s, wo_s = [], []
        for k in range(NK):
            wg_s.append(wg_pool.tile([128, HID], BF16, tag=f"wg{k}",
                                     name=f"wg_{k}"))
            wo_s.append(wo_pool.tile([128, HID], BF16, tag=f"wo{k}",
                                     name=f"wo_{k}"))

        def load_rest():
            nc.gpsimd.dma_start(densel_s[:], densel.ap()[:, :])
            nc.gpsimd.dma_start(bcsel_s[:], bcsel.ap()[:, :])
            nc.gpsimd.dma_start(bgate_s[:], bgate_c.ap()[:, :])
            for k in range(NK):
                nc.gpsimd.dma_start(
                    wg_s[k][:], wgateT.ap()[128 * k:128 * (k + 1), :])
                nc.gpsimd.dma_start(
                    wo_s[k][:], woutT.ap()[128 * k:128 * (k + 1), :])

        state = {}

        def emit_xt(g):
            tok = slice(g * WG, (g + 1) * WG)
            xts = []
            for k in range(NK):
                xt_t = xt_pool.tile([128, WG], BF16, tag="xt", name=f"xt_{g}_{k}")
                nc.sync.dma_start(xt_t[:], xT.ap()[128 * k:128 * (k + 1), tok])
                xts.append(xt_t)
            return xts

        def emit_qkv(g, xts, ots, q1, k1, vv):
            """PE: qkv matmuls for the given od tiles; DVE/ACT: phi drains."""
            is_halo = g == 0
            for ot in ots:
                sec, oti = divmod(ot, NH)
                ps = ps_pool.tile([128, WG], F32, tag="mm", name=f"qkvp_{g}_{ot}")
                for k in range(NK):
                    nc.tensor.matmul(
                        ps[:], wq_sec[sec][k][:, 128 * oti:128 * (oti + 1)],
                        xts[k][:], start=(k == 0), stop=(k == NK - 1))
                if ot < 2 * NH:  # q or k: phi drain via single psum copy
                    qc = tmp_pool.tile([128, WG], BF16, tag="qc", bufs=2,
                                       name=f"qc_{g}_{ot}")
                    nc.scalar.copy(qc[:], ps[:])
                    qm = tmp_pool.tile([128, WG], BF16, tag="phim", bufs=2,
                                       name=f"qm_{g}_{ot}")
                    nc.vector.tensor_scalar_min(qm[:], qc[:], 0.0)
                    qe = tmp_pool.tile([128, WG], BF16, tag="phie", bufs=2,
                                       name=f"qe_{g}_{ot}")
                    nc.scalar.activation(qe[:], qm[:], AF.Exp)
                    if ot < NH:
                        j = ot
                        q1[j] = qkv_pool.tile([128, WG], BF16, tag="q1",
                                              name=f"q1_{g}_{j}")
                        nc.vector.scalar_tensor_tensor(
                            q1[j][:], qc[:], 0.0, qe[:], AL.max, AL.add)
                    elif is_halo:
                        j = ot - NH
                        kr = tmp_pool.tile([128, WG], BF16, tag="kraw", bufs=1,
                                           name=f"kr_{g}_{j}")
                        nc.vector.scalar_tensor_tensor(
                            kr[:], qc[:], 0.0, qe[:], AL.max, AL.add)
                        k1[j] = qkv_pool.tile([128, WG], BF16, tag="k1",
                                              name=f"k1_{g}_{j}")
                        nc.vector.tensor_scalar_mul(
                            k1[j][:], kr[:], mask_s[:, 0:1])
                    else:
                        j = ot - NH
                        k1[j] = qkv_pool.tile([128, WG], BF16, tag="k1",
                                              name=f"k1_{g}_{j}")
                        nc.vector.scalar_tensor_tensor(
                            k1[j][:], qc[:], 0.0, qe[:], AL.max, AL.add)
                else:  # v
                    j = ot - 2 * NH
                    vv[j] = qkv_pool.tile([128, WG], BF16, tag="v", bufs=9,
                                          name=f"v_{g}_{j}")
                    nc.scalar.copy(vv[j][:], ps[:])

        def emit_oa_dl(g, q1, cum_kv, den_i, vv):
            """qckv mults, bc broadcast matmuls, attention out, and the
            (oa - v) delta — after which v is dead."""
            qckv = [None] * NH
            for j in range(NH):
                qckv[j] = tmp_pool.tile([128, WG], BF16, tag="qckv", bufs=2,
                                        name=f"qckv_{g}_{j}")
                nc.vector.tensor_mul(qckv[j][:], q1[j][:], cum_kv[j][:])
            oa = [None] * NH
            dls = [None] * NH
            for j in range(NH):
                bc = ps_pool.tile([128, WG], F32, tag="mm", name=f"bc_{g}_{j}")
                nc.tensor.matmul(
                    bc[:], bcsel_s[:, 128 * j:128 * (j + 1)], den_i[:, :],
                    start=True, stop=True)
                oa[j] = oa_pool.tile([128, WG], BF16, tag="oa",
                                     name=f"oa_{g}_{j}")
                nc.vector.tensor_mul(oa[j][:], qckv[j][:], bc[:])
                dls[j] = tmp_pool.tile([128, WG], BF16, tag="dl", bufs=9,
                                       name=f"dl_{g}_{j}")
                nc.gpsimd.tensor_sub(dls[j][:], oa[j][:], vv[j][:])
            return oa, dls

        def emit_gate(g, oa):
            gts = [None] * NH
            for ot in range(NH):
                ps = ps_pool.tile([128, WG], F32, tag="mm", name=f"gp_{g}_{ot}")
                for k in range(NK):
                    nc.tensor.matmul(
                        ps[:], wg_s[k][:, 128 * ot:128 * (ot + 1)], oa[k][:],
                        start=(k == 0), stop=(k == NK - 1))
                gts[ot] = gt_pool.tile([128, WG], BF16, tag="gt",
                                       name=f"gt_{g}_{ot}")
                nc.scalar.activation(
                    gts[ot][:], ps[:], AF.Sigmoid, bias=bgate_s[:, ot:ot + 1])
            return gts

        def emit_mix(g, gts, dls, oa):
            # mix = g*oa + (1-g)*v = (g-1)*(oa-v) + oa = (gt-1)*dl + oa
            mix = [None] * NH
            for ot in range(NH):
                d2 = tmp_pool.tile([128, WG], BF16, tag="gd",
                                   name=f"d2_{g}_{ot}")
                nc.vector.scalar_tensor_tensor(
                    d2[:], gts[ot][:], -1.0, dls[ot][:], AL.add, AL.mult)
                mix[ot] = mix_pool.tile([128, WG], BF16, tag="mix",
                                        name=f"mix_{g}_{ot}")
                nc.vector.tensor_add(mix[ot][:], d2[:], oa[ot][:])
            return mix

        def emit_y(g, mix):
            out_tok = slice(g * WG - HALO, g * WG - HALO + WG)
            for ot in range(NH):
                ps = ps_pool.tile([128, WG], F32, tag="mm", name=f"yp_{g}_{ot}")
                for k in range(NK):
                    nc.tensor.matmul(
                        ps[:], wo_s[k][:, 128 * ot:128 * (ot + 1)], mix[k][:],
                        start=(k == 0), stop=(k == NK - 1))
                ysb = y_pool.tile([128, WG], F32, tag="ysb",
                                  name=f"ysb_{g}_{ot}")
                nc.scalar.copy(ysb[:], ps[:])
                nc.sync.dma_start(
                    yT.ap()[128 * ot:128 * (ot + 1), out_tok], ysb[:])

        def emit_ksum_scans(g, k1, q1):
            """ksum scans + prod tiles: emitted right after the q-section so
            the den chain completes early in the iteration."""
            cum_ks = [None] * NH
            for j in range(NH):
                dec_b = dec_s[:, j:j + 1].broadcast_to([128, WG])
                cum_ks[j] = cum_pool.tile([128, WG], BF16, tag=f"cks{j}",
                                          name=f"cks_{g}_{j}")
                init_ks = 0.0 if g == 0 else state["ks"][j][:, 0:1]
                nc.vector.tensor_tensor_scan(
                    cum_ks[j][:], dec_b, k1[j][:], init_ks, AL.mult, AL.add)
            prods = [None] * NH
            if q1[0] is not None:
                for j in range(NH):
                    prods[j] = tmp_pool.tile([128, WG], BF16, tag="prod",
                                             bufs=9, name=f"prod_{g}_{j}")
                    nc.vector.tensor_mul(prods[j][:], q1[j][:], cum_ks[j][:])
            nks = [None] * NH
            if g < NG - 1:
                for j in range(NH):
                    nks[j] = st_pool.tile([128, 1], F32, tag=f"sks{j}",
                                          name=f"sks_{g}_{j}")
                    nc.gpsimd.tensor_copy(nks[j][:], cum_ks[j][:, WG - 1:WG])
            state["ks"] = nks
            return cum_ks, prods

        def emit_kv_scans(g, k1, vv):
            cum_kv = [None] * NH
            kvs = [None] * NH
            for j in range(NH):
                kvs[j] = tmp_pool.tile([128, WG], BF16, tag="kvp", bufs=2,
                                       name=f"kv_{g}_{j}")
                nc.gpsimd.tensor_mul(kvs[j][:], k1[j][:], vv[j][:])
            for j in range(NH):
                dec_b = dec_s[:, j:j + 1].broadcast_to([128, WG])
                cum_kv[j] = cum_pool.tile([128, WG], BF16, tag=f"ckv{j}",
                                          name=f"ckv_{g}_{j}")
                init_kv = 0.0 if g == 0 else state["kv"][j][:, 0:1]
                nc.vector.tensor_tensor_scan(
                    cum_kv[j][:], dec_b, kvs[j][:], init_kv, AL.mult, AL.add)
            nkv = [None] * NH
            if g < NG - 1:
                for j in range(NH):
                    nkv[j] = st_pool.tile([128, 1], F32, tag=f"skv{j}",
                                          name=f"skv_{g}_{j}")
                    nc.gpsimd.tensor_copy(nkv[j][:], cum_kv[j][:, WG - 1:WG])
            state["kv"] = nkv
            return cum_kv

        def emit_den(g, prods):
            dps = psd_pool.tile([H, WG], F32, tag="den", name=f"dps_{g}")
            for j in range(NH):
                nc.tensor.matmul(
                    dps[:], densel_s[:, H * j:H * (j + 1)], prods[j][:],
                    start=(j == 0), stop=(j == NH - 1))
            den_r = tmp_pool.tile([H, WG], F32, tag="denr", name=f"denr_{g}")
            nc.vector.tensor_scalar_max(den_r[:], dps[:], 1e-6)
            den_i = tmp_pool.tile([H, WG], mybir.dt.float32r, tag="deni",
                                  name=f"deni_{g}")
            with nc.allow_low_precision(reason="fp32r broadcast of reciprocal"):
                nc.vector.reciprocal(den_i[:], den_r[:])
            return den_i

        # ---- software-pipelined emission --------------------------------
        # iter g: [xt][oa/dl g-1][q g][k g][v g][gate g-1][scans g]
        #         [mix g-1][den g][y g-1]
        # The den chain for group g completes a full iteration before its
        # bc-matmul consumer; v dies at the dl subtraction so tile live
        # sets fit their pools.
        k_sec = list(range(NH, 2 * NH))
        q_sec = list(range(NH))
        v_sec = list(range(2 * NH, NOT))
        prev = None
        for g in range(NG):
            q1 = [None] * NH
            k1 = [None] * NH
            vv = [None] * NH
            xts = emit_xt(g)
            if g == 0:
                load_wq_sec(2)  # v-section, after xt g0 in queue order
                load_wq_sec(0)  # q-section next (needed ~45us in)
            emit_qkv(g, xts, k_sec, q1, k1, vv)
            if prev is not None:
                p_q1, p_ckv, p_vv, p_den, pg = prev
                oa, dls = emit_oa_dl(pg, p_q1, p_ckv, p_den, p_vv)
            if g > 0:
                emit_qkv(g, xts, q_sec, q1, k1, vv)
            cum_ks, prods = emit_ksum_scans(g, k1, q1)
            emit_qkv(g, xts, v_sec, q1, k1, vv)
            if g == 1:
                load_rest()
            if g > 0:
                den_i = emit_den(g, prods)
            if prev is not None:
                gts = emit_gate(pg, oa)
            cum_kv = emit_kv_scans(g, k1, vv)
            if prev is not None:
                mix = emit_mix(pg, gts, dls, oa)
                emit_y(pg, mix)
            if g > 0:
                prev = (q1, cum_kv, vv, den_i, g)
        q1, cum_kv, vv, den_i, g = prev
        oa, dls = emit_oa_dl(g, q1, cum_kv, den_i, vv)
        gts = emit_gate(g, oa)
        mix = emit_mix(g, gts, dls, oa)
        emit_y(g, mix)

    nc.compile()
    return nc


def _sigmoid(v):
    return 1.0 / (1.0 + np.exp(-v))


def _make_inputs(x, Wqkv, Wout, Wgate, bgate, decay_param):
    decay = _sigmoid(np.asarray(decay_param, np.float64)).astype(np.float32)
    bf = ml_dtypes.bfloat16
    wqkvT = np.ascontiguousarray(np.asarray(Wqkv, np.float32).T).astype(bf)
    wgateT = np.ascontiguousarray(np.asarray(Wgate, np.float32).T).astype(bf)
    woutT = np.ascontiguousarray(np.asarray(Wout, np.float32).T).astype(bf)

    p = np.arange(128)
    dec_c = np.empty((128, NH), np.float32)
    for j in range(NH):
        dec_c[:, j] = decay[2 * j + p // 64]
    densel = np.zeros((128, NH * H), np.float32)
    for j in range(NH):
        for pp in range(128):
            densel[pp, H * j + 2 * j + pp // 64] = 1.0
    bcsel = np.zeros((H, NH * 128), np.float32)
    for j in range(NH):
        for m in range(128):
            bcsel[2 * j + m // 64, 128 * j + m] = 1.0
    bgate_c = np.ascontiguousarray(
        np.asarray(bgate, np.float32).reshape(NH, 128).T)

    in_maps = []
    for c in range(8):
        b, half = c // 2, c % 2
        xb = np.asarray(x[b], np.float32)  # [T, HID]
        if half == 0:
            xloc = np.concatenate(
                [np.zeros((HALO, HID), np.float32), xb[:HALF_T]], axis=0)
            mask = np.zeros((128, 1), np.float32)
        else:
            xloc = xb[HALF_T - HALO:]
            mask = np.ones((128, 1), np.float32)
        in_maps.append({
            "xT": np.ascontiguousarray(xloc.T).astype(bf),
            "wqkvT": wqkvT, "wgateT": wgateT, "woutT": woutT,
            "dec_c": dec_c, "mask_c": mask,
            "densel": densel.astype(bf), "bcsel": bcsel,
            "bgate_c": bgate_c,
        })
    return in_maps


def kernel(x, Wqkv, Wout, Wgate, bgate, decay_param):
    if "nc" not in _cache:
        _cache["nc"] = _build_nc()
    nc = _cache["nc"]
    in_maps = _make_inputs(x, Wqkv, Wout, Wgate, bgate, decay_param)
    res = run_bass_kernel_spmd(nc, in_maps, list(range(8)))
    y = np.empty((B, T, HID), np.float32)
    for c in range(8):
        b, half = c // 2, c % 2
        y[b, half * HALF_T:(half + 1) * HALF_T, :] = res.results[c]["yT"].T
    return y



# revision 15
# speedup vs baseline: 1.1814x; 1.1814x over previous
"""Trainium2 Bass kernel for nn_LinearAttention (gated linear attention).

Math (per reference):
    qkv = x @ Wqkv.T ; q,k,v = split(qkv); q,k = elu(.)+1
    per (b,h): running_kv[t]  = d*running_kv[t-1]  + k[t]*v[t]   (elementwise, D=64)
               running_ksum[t]= d*running_ksum[t-1]+ k[t]
    den = clip(sum_d(q*running_ksum), 1e-6); out = q*running_kv/den
    g = sigmoid(out @ Wgate.T + bgate); out = g*out + (1-g)*v
    y = out @ Wout.T

Implementation strategy (8 NeuronCores, SPMD, no collectives):
  - Token-parallel: core c handles batch b=c//2, T-half h=c%2 (2048 tokens)
    plus a 128-token halo to warm the decay scan (0.95^128 ~ 1.4e-3, well
    under the error budget).  Half 0 gets a zero halo + k-mask so its scan
    state is exactly 0 at t=0.
  - Everything on-chip is [feature(partition), token(free)]; the host
    pre-transposes x and the weights so no on-chip transpose is needed.
  - All activations ride a x32 scale (from phi's drain identity
    32*phi(x) = max(ps+32, 32*min(exp(ps/32),1)) with ps = 32x unscaled
    psum... ps here is the raw qkv psum; the pipeline is linear/ratio in
    the scale so it cancels everywhere except the gate sigmoid (ACT
    scale=1/128) and the final y copy (ACT scale=1/32), both free.
  - The gate matmul runs in fp8 DoubleRow (2x PE rate); its quantization
    error is damped by the sigmoid (verified in simulation).  qkv and out
    matmuls stay bf16 for accuracy.
  - Decay scans run on the Vector engine via tensor_tensor_scan; iter 1
    scans are 640 wide (128 halo + 512), later iters 512, chained via
    [128,1] state tiles.
  - den: 0/1 block-diag selector matmul -> [16,512] psum; clip, fast
    approx reciprocal, cast bf16; broadcast back to 128 partitions via a
    bf16 selector matmul.
  - gate/out matmuls for iter i run in iter i+1 (software pipeline), so
    every PE dependency is a full iteration old.
"""

import sys

for _p in ('/opt/trn_rl_repo', '/root/.axon_site'):
    if _p not in sys.path:
        sys.path.insert(0, _p)

from contextlib import ExitStack

import ml_dtypes
import numpy as np

import concourse.tile as tile
from concourse import bacc, mybir
from concourse.bass_utils import run_bass_kernel_spmd

F32 = mybir.dt.float32
BF16 = mybir.dt.bfloat16
FP8 = mybir.dt.float8e4
AL = mybir.AluOpType
AF = mybir.ActivationFunctionType
DR = mybir.MatmulPerfMode.DoubleRow

B, T, HID = 4, 4096, 1024
H, D = 16, 64
OD = 3 * HID
NK = HID // 128            # 8 contraction tiles
NH = HID // 128            # 8 tiles per q/k/v section
HALF_T = T // 2            # 2048 out tokens per core
HALO = 128
TLOC = HALO + HALF_T       # 2176
WG = 512                   # out-token group width
NG = HALF_T // WG          # 4 iterations
W1 = HALO + WG             # 640: iter-1 scan width

S = 32.0                   # activation scale riding the pipeline
OSC = 1.0 / 8.0            # oa -> fp8 cast scale (4*att, safely < 240)
GS = 1.0 / (S * S * OSC)   # gate sigmoid descale = 1/128

_cache = {}


def _build_nc():
    nc = bacc.Bacc("TRN2", target_bir_lowering=False, debug=False)

    xT = nc.dram_tensor("xT", [HID, TLOC], BF16, kind="ExternalInput")
    wqkvT = nc.dram_tensor("wqkvT", [HID, OD], BF16, kind="ExternalInput")
    wg8 = nc.dram_tensor("wg8", [HID, HID], FP8, kind="ExternalInput")
    woutT = nc.dram_tensor("woutT", [HID, HID], BF16, kind="ExternalInput")
    dec_c = nc.dram_tensor("dec_c", [128, NH], F32, kind="ExternalInput")
    mask_c = nc.dram_tensor("mask_c", [128, 1], F32, kind="ExternalInput")
    densel = nc.dram_tensor("densel", [128, NH * H], BF16, kind="ExternalInput")
    bcsel = nc.dram_tensor("bcsel", [H, NH * 128], BF16, kind="ExternalInput")
    bgate_c = nc.dram_tensor("bgate_c", [128, NH], F32, kind="ExternalInput")
    yT = nc.dram_tensor("yT", [HID, HALF_T], F32, kind="ExternalOutput")

    with tile.TileContext(nc) as tc, ExitStack() as ctx:
        consts = ctx.enter_context(tc.tile_pool(name="consts", bufs=1))
        wq_pool = ctx.enter_context(tc.tile_pool(name="wq", bufs=1))
        wg_pool = ctx.enter_context(tc.tile_pool(name="wgp", bufs=1))
        wo_pool = ctx.enter_context(tc.tile_pool(name="wop", bufs=1))
        xt_pool = ctx.enter_context(tc.tile_pool(name="xt", bufs=9))
        k1_pool = ctx.enter_context(tc.tile_pool(name="k1p", bufs=8))
        q1_pool = ctx.enter_context(tc.tile_pool(name="q1p", bufs=8))
        v1_pool = ctx.enter_context(tc.tile_pool(name="v1p", bufs=16))
        et_pool = ctx.enter_context(tc.tile_pool(name="et", bufs=2))
        kv_pool = ctx.enter_context(tc.tile_pool(name="kvp", bufs=1))
        cum_pool = ctx.enter_context(tc.tile_pool(name="cum", bufs=1))
        st_pool = ctx.enter_context(tc.tile_pool(name="st", bufs=2))
        pr_pool = ctx.enter_context(tc.tile_pool(name="pr", bufs=8))
        den_pool = ctx.enter_context(tc.tile_pool(name="den", bufs=1))
        oa_pool = ctx.enter_context(tc.tile_pool(name="oa", bufs=2))
        oa8_pool = ctx.enter_context(tc.tile_pool(name="oa8", bufs=2))
        dl_pool = ctx.enter_context(tc.tile_pool(name="dl", bufs=10))
        gt_pool = ctx.enter_context(tc.tile_pool(name="gt", bufs=2))
        mx_pool = ctx.enter_context(tc.tile_pool(name="mx", bufs=8))
        y_pool = ctx.enter_context(tc.tile_pool(name="ysb", bufs=2))
        ps_pool = ctx.enter_context(tc.tile_pool(name="ps", bufs=7, space="PSUM"))
        psd_pool = ctx.enter_context(tc.tile_pool(name="psd", bufs=1, space="PSUM"))

        # ---- weight/const loads: weights on the gpsimd DMA queue, x on sync
        wq_sec = {}
        for sec in range(3):
            wq_sec[sec] = [wq_pool.tile([128, HID], BF16, tag=f"wq{sec}_{k}",
                                        name=f"wq_{sec}_{k}") for k in range(NK)]

        def load_wq_sec(sec):
            for k in range(NK):
                nc.gpsimd.dma_start(
                    wq_sec[sec][k][:],
                    wqkvT.ap()[128 * k:128 * (k + 1), HID * sec:HID * (sec + 1)])

        dec_s = consts.tile([128, NH], F32, tag="dec")
        mask_s = consts.tile([128, 1], F32, tag="mask")
        densel_s = consts.tile([128, NH * H], BF16, tag="densel")
        bcsel_s = consts.tile([H, NH * 128], BF16, tag="bcsel")
        bgate_s = consts.tile([128, NH], F32, tag="bg")
        nc.sync.dma_start(dec_s[:], dec_c.ap()[:, :])
        nc.sync.dma_start(mask_s[:], mask_c.ap()[:, :])
        nc.sync.dma_start(densel_s[:], densel.ap()[:, :])
        nc.sync.dma_start(bcsel_s[:], bcsel.ap()[:, :])
        nc.sync.dma_start(bgate_s[:], bgate_c.ap()[:, :])

        load_wq_sec(1)  # k-section first: the PE needs it immediately

        wg8_s = [wg_pool.tile([128, 2 * HID], FP8, tag=f"wg{kp}",
                              name=f"wg_{kp}") for kp in range(NK // 2)]
        wo_s = [wo_pool.tile([128, HID], BF16, tag=f"wo{k}",
                             name=f"wo_{k}") for k in range(NK)]

        def load_rest():
            for kp in range(NK // 2):
                nc.gpsimd.dma_start(
                    wg8_s[kp][:, 0:HID],
                    wg8.ap()[256 * kp:256 * kp + 128, :])
                nc.gpsimd.dma_start(
                    wg8_s[kp][:, HID:2 * HID],
                    wg8.ap()[256 * kp + 128:256 * kp + 256, :])
            for k in range(NK):
                nc.gpsimd.dma_start(
                    wo_s[k][:], woutT.ap()[128 * k:128 * (k + 1), :])

        # ---- per-iteration emission helpers ------------------------------
        def emit_x(i):
            """x tiles for iteration i (1..NG): cols [HALO+(i-1)*WG, ...)."""
            tok = slice(HALO + (i - 1) * WG, HALO + i * WG)
            xts = []
            for k in range(NK):
                t = xt_pool.tile([128, WG], BF16, tag="xt", name=f"xt_{i}_{k}")
                nc.sync.dma_start(t[:], xT.ap()[128 * k:128 * (k + 1), tok])
                xts.append(t)
            return xts

        def phi_drain(ps, dst, w, name):
            """dst[:, :w] = 32*phi(ps/32) = max(ps+32, 32*min(exp(ps/32),1))"""
            e = et_pool.tile([128, WG], BF16, tag="e", name=f"e_{name}")
            nc.scalar.activation(e[:, 0:w], ps[:, 0:w], AF.Exp, scale=1.0 / S)
            tm = et_pool.tile([128, WG], BF16, tag="tm", name=f"tm_{name}")
            nc.vector.tensor_scalar(tm[:, 0:w], e[:, 0:w], 1.0, S,
                                    AL.min, AL.mult)
            nc.vector.scalar_tensor_tensor(dst[:, 0:w], ps[:, 0:w], S,
                                           tm[:, 0:w], AL.add, AL.max)

        def emit_sec(i, sec, xts, drain, w=WG, xoff=0):
            """One qkv section: 8 psum tiles, each drained via drain(j, ps)."""
            for j in range(NH):
                oti = j
                ps = ps_pool.tile([128, WG], F32, tag="mm",
                                  name=f"ps_{i}_{sec}_{j}")
                for k in range(NK):
                    nc.tensor.matmul(
                        ps[:, 0:w],
                        wq_sec[sec][k][:, 128 * oti:128 * (oti + 1)],
                        xts[k][:, xoff:xoff + w],
                        start=(k == 0), stop=(k == NK - 1))
                drain(j, ps)

        state = {"ks": [None] * NH, "kv": [None] * NH}

        def emit_scans(i, which, data, out_w):
            """8 scans of width out_w; returns cum tiles [128, W1]."""
            cums = []
            for j in range(NH):
                dec_b = dec_s[:, j:j + 1].broadcast_to([128, out_w])
                cum = cum_pool.tile([128, W1], BF16, tag=f"c{which}{j}",
                                    name=f"c{which}_{i}_{j}")
                init = 0.0 if i == 1 else state[which][j][:, 0:1]
                nc.vector.tensor_tensor_scan(
                    cum[:, 0:out_w], dec_b, data[j][:, 0:out_w], init,
                    AL.mult, AL.add)
                cums.append(cum)
            return cums

        def emit_state(i, which, cums, out_w):
            if i == NG:
                return
            nxt = []
            for j in range(NH):
                s = st_pool.tile([128, 1], F32, tag=f"s{which}{j}",
                                 name=f"s{which}_{i}_{j}")
                nc.gpsimd.tensor_copy(s[:], cums[j][:, out_w - 1:out_w])
                nxt.append(s)
            state[which] = nxt

        def emit_gate_mix(p_oa8, p_dls, p_v1, p_i):
            """fp8 DoubleRow gate matmul, sigmoid, and mix = g*(oa-v)+v,
            interleaved per output tile to keep the gts live set at 2."""
            mixes = []
            for ot in range(NH):
                ps = ps_pool.tile([128, WG], F32, tag="mm",
                                  name=f"gp_{p_i}_{ot}")
                for kp in range(NK // 2):
                    lhs = wg8_s[kp][:, :].rearrange(
                        "p (i m) -> p i m", i=2)[:, :, 128 * ot:128 * (ot + 1)]
                    rhs = p_oa8[kp][:, :].rearrange("p (i n) -> p i n", i=2)
                    nc.tensor.matmul(ps[:], lhs, rhs, start=(kp == 0),
                                     stop=(kp == NK // 2 - 1), perf_mode=DR)
                g = gt_pool.tile([128, WG], BF16, tag="gt",
                                 name=f"gt_{p_i}_{ot}")
                nc.scalar.activation(g[:], ps[:], AF.Sigmoid,
                                     bias=bgate_s[:, ot:ot + 1], scale=GS)
                m1 = et_pool.tile([128, WG], BF16, tag="m1",
                                  name=f"m1_{p_i}_{ot}")
                nc.vector.tensor_tensor(m1[:], g[:], p_dls[ot][:], AL.mult)
                mx = mx_pool.tile([128, WG], BF16, tag="mx",
                                  name=f"mx_{p_i}_{ot}")
                nc.vector.tensor_tensor(mx[:], m1[:], p_v1[ot][:], AL.add)
                mixes.append(mx)
            return mixes

        def emit_out(mixes, p_i):
            out_tok = slice((p_i - 1) * WG, p_i * WG)
            for ot in range(NH):
                ps = ps_pool.tile([128, WG], F32, tag="mm",
                                  name=f"yp_{p_i}_{ot}")
                for k in range(NK):
                    nc.tensor.matmul(
                        ps[:], wo_s[k][:, 128 * ot:128 * (ot + 1)],
                        mixes[k][:], start=(k == 0), stop=(k == NK - 1))
                ysb = y_pool.tile([128, WG], F32, tag="ysb",
                                  name=f"ysb_{p_i}_{ot}")
                nc.scalar.activation(ysb[:], ps[:], AF.Copy, scale=1.0 / S)
                nc.sync.dma_start(yT.ap()[128 * ot:128 * (ot + 1), out_tok],
                                  ysb[:])

        # ================= prologue: halo ================================
        xh = []
        for k in range(NK):
            t = xt_pool.tile([128, HALO], BF16, tag="xh", bufs=8,
                             name=f"xh_{k}")
            nc.sync.dma_start(t[:], xT.ap()[128 * k:128 * (k + 1), 0:HALO])
            xh.append(t)
        x1 = emit_x(1)
        load_wq_sec(2)  # v-section
        load_wq_sec(0)  # q-section

        k1_1 = [k1_pool.tile([128, W1], BF16, tag="k1", name=f"k1_1_{j}")
                for j in range(NH)]
        kvs_1 = [kv_pool.tile([128, W1], BF16, tag=f"kv{j}", name=f"kvs_1_{j}")
                 for j in range(NH)]

        for j in range(NH):  # halo k-section
            ps = ps_pool.tile([128, WG], F32, tag="mm", name=f"psh_k_{j}")
            for k in range(NK):
                nc.tensor.matmul(ps[:, 0:HALO],
                                 wq_sec[1][k][:, 128 * j:128 * (j + 1)],
                                 xh[k][:, 0:HALO],
                                 start=(k == 0), stop=(k == NK - 1))
            kr = et_pool.tile([128, HALO], BF16, tag="kr", name=f"krh_{j}")
            phi_drain(ps, kr, HALO, f"kh_{j}")
            # mask: half-0 cores zero the halo k so scan state starts at 0
            nc.vector.tensor_scalar_mul(k1_1[j][:, 0:HALO], kr[:, 0:HALO],
                                        mask_s[:, 0:1])
        for j in range(NH):  # halo v-section; k*v goes straight to kvs_1
            ps = ps_pool.tile([128, WG], F32, tag="mm", name=f"psh_v_{j}")
            for k in range(NK):
                nc.tensor.matmul(ps[:, 0:HALO],
                                 wq_sec[2][k][:, 128 * j:128 * (j + 1)],
                                 xh[k][:, 0:HALO],
                                 start=(k == 0), stop=(k == NK - 1))
            vh = et_pool.tile([128, HALO], BF16, tag="vh", name=f"vh_{j}")
            nc.scalar.copy(vh[:], ps[:, 0:HALO])
            nc.gpsimd.tensor_tensor(kvs_1[j][:, 0:HALO], k1_1[j][:, 0:HALO],
                                    vh[:], AL.mult)
        load_rest()

        # ================= main loop =====================================
        prev = None  # (oa8_pairs, dls, v1, i)
        xs = {1: x1}
        for i in range(1, NG + 1):
            koff = HALO if i == 1 else 0
            w1 = W1 if i == 1 else WG
            xts = xs.pop(i)
            if i < NG:
                xs[i + 1] = emit_x(i + 1)

            if i == 1:
                k1_i, kvs_i = k1_1, kvs_1
            else:
                k1_i = [k1_pool.tile([128, W1], BF16, tag="k1",
                                     name=f"k1_{i}_{j}") for j in range(NH)]
                kvs_i = [kv_pool.tile([128, W1], BF16, tag=f"kv{j}",
                                      name=f"kvs_{i}_{j}") for j in range(NH)]
            v1_i = [v1_pool.tile([128, WG], BF16, tag="v1",
                                 name=f"v1_{i}_{j}") for j in range(NH)]

            # PE: k-section; DVE/ACT: phi-k drains
            def drain_k(j, ps, k1_i=k1_i, koff=koff, i=i):
                # write into [koff : koff+WG]
                kview = k1_i[j][:, koff:koff + WG]
                e = et_pool.tile([128, WG], BF16, tag="e", name=f"e_k{i}_{j}")
                nc.scalar.activation(e[:], ps[:], AF.Exp, scale=1.0 / S)
                tm = et_pool.tile([128, WG], BF16, tag="tm",
                                  name=f"tm_k{i}_{j}")
                nc.vector.tensor_scalar(tm[:], e[:], 1.0, S, AL.min, AL.mult)
                nc.vector.scalar_tensor_tensor(kview, ps[:], S, tm[:],
                                               AL.add, AL.max)
            emit_sec(i, 1, xts, drain_k)

            # PE: q-section; phi-q drains
            q1_i = [q1_pool.tile([128, WG], BF16, tag="q1",
                                 name=f"q1_{i}_{j}") for j in range(NH)]

            def drain_q(j, ps, q1_i=q1_i, i=i):
                e = et_pool.tile([128, WG], BF16, tag="e", name=f"e_q{i}_{j}")
                nc.scalar.activation(e[:], ps[:], AF.Exp, scale=1.0 / S)
                tm = et_pool.tile([128, WG], BF16, tag="tm",
                                  name=f"tm_q{i}_{j}")
                nc.vector.tensor_scalar(tm[:], e[:], 1.0, S, AL.min, AL.mult)
                nc.vector.scalar_tensor_tensor(q1_i[j][:], ps[:], S, tm[:],
                                               AL.add, AL.max)
            emit_sec(i, 0, xts, drain_q)

            # PE: gate matmul for prev iter (fp8 DoubleRow); sigmoid + mix
            if prev is not None:
                p_oa8, p_dls, p_v1, p_i = prev
                mixes = emit_gate_mix(p_oa8, p_dls, p_v1, p_i)

            # DVE: ksum scans + prods
            cks = emit_scans(i, "ks", k1_i, w1)
            prods = []
            for j in range(NH):
                pr = pr_pool.tile([128, WG], BF16, tag="pr",
                                  name=f"pr_{i}_{j}")
                nc.vector.tensor_tensor(pr[:], q1_i[j][:],
                                        cks[j][:, koff:koff + WG], AL.mult)
                prods.append(pr)
            emit_state(i, "ks", cks, w1)

            # PE: v-section; ACT copies
            def drain_v(j, ps, v1_i=v1_i):
                nc.scalar.copy(v1_i[j][:], ps[:])
            emit_sec(i, 2, xts, drain_v)

            # gpsimd: k*v products
            for j in range(NH):
                nc.gpsimd.tensor_tensor(kvs_i[j][:, koff:koff + WG],
                                        k1_i[j][:, koff:koff + WG],
                                        v1_i[j][:], AL.mult)

            # PE: den selector matmul; DVE: clip (in-place), recip, cast
            dps = psd_pool.tile([H, WG], F32, tag="den", name=f"dps_{i}")
            for j in range(NH):
                nc.tensor.matmul(dps[:], densel_s[:, H * j:H * (j + 1)],
                                 prods[j][:], start=(j == 0),
                                 stop=(j == NH - 1))
            nc.vector.tensor_scalar_max(dps[:], dps[:], 1e-6 * S * S)
            den_f = den_pool.tile([H, WG], F32, tag="denf", name=f"denf_{i}")
            nc.vector.reciprocal_approx_fast(den_f[:], dps[:])
            den_i = den_pool.tile([H, WG], BF16, tag="deni", name=f"deni_{i}")
            nc.vector.tensor_scalar_mul(den_i[:], den_f[:], 1.0)

            # DVE: kv scans
            ckv = emit_scans(i, "kv", kvs_i, w1)
            emit_state(i, "kv", ckv, w1)

            # PE: bc broadcast matmul (bf16)
            bcs = []
            for j in range(NH):
                bc = ps_pool.tile([128, WG], F32, tag="mm",
                                  name=f"bc_{i}_{j}")
                nc.tensor.matmul(bc[:], bcsel_s[:, 128 * j:128 * (j + 1)],
                                 den_i[:, :], start=True, stop=True)
                bcs.append(bc)

            # DVE: qckv, oa, oa8; gpsimd: dls
            oa8 = [oa8_pool.tile([128, 2 * WG], FP8, tag=f"o8{kp}",
                                 name=f"oa8_{i}_{kp}")
                   for kp in range(NH // 2)]
            dls = []
            for j in range(NH):
                qc = et_pool.tile([128, WG], BF16, tag="qc", bufs=2,
                                  name=f"qc_{i}_{j}")
                nc.vector.tensor_tensor(qc[:], q1_i[j][:],
                                        ckv[j][:, koff:koff + WG], AL.mult)
                oa = oa_pool.tile([128, WG], BF16, tag="oa",
                                  name=f"oa_{i}_{j}")
                nc.vector.tensor_tensor(oa[:], qc[:], bcs[j][:], AL.mult)
                nc.vector.tensor_scalar_mul(
                    oa8[j // 2][:, WG * (j % 2):WG * (j % 2 + 1)],
                    oa[:], OSC)
                dl = dl_pool.tile([128, WG], BF16, tag="dl",
                                  name=f"dl_{i}_{j}")
                nc.gpsimd.tensor_tensor(dl[:], oa[:], v1_i[j][:], AL.subtract)
                dls.append(dl)

            # PE: out matmul for prev iter; ACT y copies; DMA out
            if prev is not None:
                emit_out(mixes, p_i)

            prev = (oa8, dls, v1_i, i)

        # ================= epilogue: gate/mix/out for last iter ==========
        p_oa8, p_dls, p_v1, p_i = prev
        mixes = emit_gate_mix(p_oa8, p_dls, p_v1, p_i)
        emit_out(mixes, p_i)

    nc.compile()
    return nc


def _sigmoid(v):
    return 1.0 / (1.0 + np.exp(-v))


def _make_inputs(x, Wqkv, Wout, Wgate, bgate, decay_param):
    decay = _sigmoid(np.asarray(decay_param, np.float64)).astype(np.float32)
    bf = ml_dtypes.bfloat16
    f8 = ml_dtypes.float8_e4m3
    # x32: the whole pipeline rides this scale (see module docstring);
    # scaling by a power of two is exact in bf16.
    wqkvT = np.ascontiguousarray(
        np.asarray(Wqkv, np.float32).T * np.float32(S)).astype(bf)
    wg8 = np.ascontiguousarray(
        np.asarray(Wgate, np.float32).T * np.float32(S)).astype(f8)
    woutT = np.ascontiguousarray(np.asarray(Wout, np.float32).T).astype(bf)

    p = np.arange(128)
    dec_c = np.empty((128, NH), np.float32)
    for j in range(NH):
        dec_c[:, j] = decay[2 * j + p // 64]
    densel = np.zeros((128, NH * H), np.float32)
    for j in range(NH):
        for pp in range(128):
            densel[pp, H * j + 2 * j + pp // 64] = 1.0
    bcsel = np.zeros((H, NH * 128), np.float32)
    for j in range(NH):
        for m in range(128):
            bcsel[2 * j + m // 64, 128 * j + m] = 1.0
    bgate_c = np.ascontiguousarray(
        np.asarray(bgate, np.float32).reshape(NH, 128).T)

    in_maps = []
    for c in range(8):
        b, half = c // 2, c % 2
        xb = np.asarray(x[b], np.float32)  # [T, HID]
        if half == 0:
            xloc = np.concatenate(
                [np.zeros((HALO, HID), np.float32), xb[:HALF_T]], axis=0)
            mask = np.zeros((128, 1), np.float32)
        else:
            xloc = xb[HALF_T - HALO:]
            mask = np.ones((128, 1), np.float32)
        in_maps.append({
            "xT": np.ascontiguousarray(xloc.T).astype(bf),
            "wqkvT": wqkvT, "wg8": wg8, "woutT": woutT,
            "dec_c": dec_c, "mask_c": mask,
            "densel": densel.astype(bf), "bcsel": bcsel.astype(bf),
            "bgate_c": bgate_c,
        })
    return in_maps


def kernel(x, Wqkv, Wout, Wgate, bgate, decay_param):
    if "nc" not in _cache:
        _cache["nc"] = _build_nc()
    nc = _cache["nc"]
    in_maps = _make_inputs(x, Wqkv, Wout, Wgate, bgate, decay_param)
    res = run_bass_kernel_spmd(nc, in_maps, list(range(8)))
    y = np.empty((B, T, HID), np.float32)
    for c in range(8):
        b, half = c // 2, c % 2
        y[b, half * HALF_T:(half + 1) * HALF_T, :] = res.results[c]["yT"].T
    return y


# revision 18
# speedup vs baseline: 1.2781x; 1.0818x over previous
"""Trainium2 Bass kernel for nn_LinearAttention (gated linear attention).

Math (per reference):
    qkv = x @ Wqkv.T ; q,k,v = split(qkv); q,k = elu(.)+1
    per (b,h): running_kv[t]  = d*running_kv[t-1]  + k[t]*v[t]   (elementwise, D=64)
               running_ksum[t]= d*running_ksum[t-1]+ k[t]
    den = clip(sum_d(q*running_ksum), 1e-6); out = q*running_kv/den
    g = sigmoid(out @ Wgate.T + bgate); out = g*out + (1-g)*v
    y = out @ Wout.T

Implementation strategy (8 NeuronCores, SPMD, no collectives):
  - Token-parallel: core c handles batch b=c//2, T-half h=c%2 (2048 tokens)
    plus a 128-token halo to warm the decay scan (0.95^128 ~ 1.4e-3, well
    under the error budget).  Half 0 gets a zero halo + k-mask so its scan
    state is exactly 0 at t=0.
  - Everything on-chip is [feature(partition), token(free)]; the host
    pre-transposes x and the weights so no on-chip transpose is needed.
  - All activations ride a x32 scale (from phi's drain identity
    32*phi(x) = max(ps+32, 32*min(exp(ps/32),1)) with ps = 32x unscaled
    psum... ps here is the raw qkv psum; the pipeline is linear/ratio in
    the scale so it cancels everywhere except the gate sigmoid (ACT
    scale=1/128) and the final y copy (ACT scale=1/32), both free.
  - The gate matmul runs in fp8 DoubleRow (2x PE rate); its quantization
    error is damped by the sigmoid (verified in simulation).  qkv and out
    matmuls stay bf16 for accuracy.
  - Decay scans run on the Vector engine via tensor_tensor_scan; iter 1
    scans are 640 wide (128 halo + 512), later iters 512, chained via
    [128,1] state tiles.
  - den: 0/1 block-diag selector matmul -> [16,512] psum; clip, fast
    approx reciprocal, cast bf16; broadcast back to 128 partitions via a
    bf16 selector matmul.
  - gate/out matmuls for iter i run in iter i+1 (software pipeline), so
    every PE dependency is a full iteration old.
"""

import sys

for _p in ('/opt/trn_rl_repo', '/root/.axon_site'):
    if _p not in sys.path:
        sys.path.insert(0, _p)

from contextlib import ExitStack

import ml_dtypes
import numpy as np

import concourse.tile as tile
from concourse import bacc, mybir
from concourse.bass_utils import run_bass_kernel_spmd

F32 = mybir.dt.float32
BF16 = mybir.dt.bfloat16
FP8 = mybir.dt.float8e4
AL = mybir.AluOpType
AF = mybir.ActivationFunctionType
DR = mybir.MatmulPerfMode.DoubleRow

B, T, HID = 4, 4096, 1024
H, D = 16, 64
OD = 3 * HID
NK = HID // 128            # 8 contraction tiles
NH = HID // 128            # 8 tiles per q/k/v section
HALF_T = T // 2            # 2048 out tokens per core
HALO = 128
TLOC = HALO + HALF_T       # 2176
WG = 512                   # out-token group width
NG = HALF_T // WG          # 4 iterations
W1 = HALO + WG             # 640: iter-1 scan width

S = 32.0                   # activation scale riding the pipeline
OSC = 1.0 / 8.0            # oa -> fp8 cast scale (4*att, safely < 240)
GS = 1.0 / (S * S * OSC)   # gate sigmoid descale = 1/128

_cache = {}


def _build_nc():
    nc = bacc.Bacc("TRN2", target_bir_lowering=False, debug=False)

    xT = nc.dram_tensor("xT", [HID, TLOC], BF16, kind="ExternalInput")
    wqkvT = nc.dram_tensor("wqkvT", [HID, OD], BF16, kind="ExternalInput")
    wg8 = nc.dram_tensor("wg8", [HID, HID], FP8, kind="ExternalInput")
    woutT = nc.dram_tensor("woutT", [HID, HID], BF16, kind="ExternalInput")
    dec_c = nc.dram_tensor("dec_c", [128, NH], F32, kind="ExternalInput")
    mask_c = nc.dram_tensor("mask_c", [128, 1], F32, kind="ExternalInput")
    densel = nc.dram_tensor("densel", [128, NH * H], BF16, kind="ExternalInput")
    bcsel = nc.dram_tensor("bcsel", [H, NH * 128], BF16, kind="ExternalInput")
    bgate_c = nc.dram_tensor("bgate_c", [128, NH], F32, kind="ExternalInput")
    yT = nc.dram_tensor("yT", [HID, HALF_T], F32, kind="ExternalOutput")

    with tile.TileContext(nc) as tc, ExitStack() as ctx:
        consts = ctx.enter_context(tc.tile_pool(name="consts", bufs=1))
        wq_pool = ctx.enter_context(tc.tile_pool(name="wq", bufs=1))
        wg_pool = ctx.enter_context(tc.tile_pool(name="wgp", bufs=1))
        wo_pool = ctx.enter_context(tc.tile_pool(name="wop", bufs=1))
        xt_pool = ctx.enter_context(tc.tile_pool(name="xt", bufs=9))
        k1_pool = ctx.enter_context(tc.tile_pool(name="k1p", bufs=8))
        q1_pool = ctx.enter_context(tc.tile_pool(name="q1p", bufs=8))
        v1_pool = ctx.enter_context(tc.tile_pool(name="v1p", bufs=16))
        et_pool = ctx.enter_context(tc.tile_pool(name="et", bufs=2))
        kv_pool = ctx.enter_context(tc.tile_pool(name="kvp", bufs=1))
        cum_pool = ctx.enter_context(tc.tile_pool(name="cum", bufs=1))
        st_pool = ctx.enter_context(tc.tile_pool(name="st", bufs=2))
        pr_pool = ctx.enter_context(tc.tile_pool(name="pr", bufs=8))
        den_pool = ctx.enter_context(tc.tile_pool(name="den", bufs=1))
        oa_pool = ctx.enter_context(tc.tile_pool(name="oa", bufs=2))
        oa8_pool = ctx.enter_context(tc.tile_pool(name="oa8", bufs=2))
        dl_pool = ctx.enter_context(tc.tile_pool(name="dl", bufs=10))
        gt_pool = ctx.enter_context(tc.tile_pool(name="gt", bufs=2))
        mx_pool = ctx.enter_context(tc.tile_pool(name="mx", bufs=8))
        y_pool = ctx.enter_context(tc.tile_pool(name="ysb", bufs=2))
        ps_pool = ctx.enter_context(tc.tile_pool(name="ps", bufs=7, space="PSUM"))
        psd_pool = ctx.enter_context(tc.tile_pool(name="psd", bufs=1, space="PSUM"))

        # ---- weight/const loads: weights on the gpsimd DMA queue, x on sync
        wq_sec = {}
        for sec in range(3):
            wq_sec[sec] = [wq_pool.tile([128, HID], BF16, tag=f"wq{sec}_{k}",
                                        name=f"wq_{sec}_{k}") for k in range(NK)]

        def load_wq_sec(sec):
            for k in range(NK):
                nc.gpsimd.dma_start(
                    wq_sec[sec][k][:],
                    wqkvT.ap()[128 * k:128 * (k + 1), HID * sec:HID * (sec + 1)])

        dec_s = consts.tile([128, NH], F32, tag="dec")
        mask_s = consts.tile([128, 1], F32, tag="mask")
        densel_s = consts.tile([128, NH * H], BF16, tag="densel")
        bcsel_s = consts.tile([H, NH * 128], BF16, tag="bcsel")
        bgate_s = consts.tile([128, NH], F32, tag="bg")
        nc.sync.dma_start(dec_s[:], dec_c.ap()[:, :])
        nc.sync.dma_start(mask_s[:], mask_c.ap()[:, :])
        nc.sync.dma_start(densel_s[:], densel.ap()[:, :])
        nc.sync.dma_start(bcsel_s[:], bcsel.ap()[:, :])
        nc.sync.dma_start(bgate_s[:], bgate_c.ap()[:, :])

        load_wq_sec(1)  # k-section first: the PE needs it immediately

        wg8_s = [wg_pool.tile([128, 2 * HID], FP8, tag=f"wg{kp}",
                              name=f"wg_{kp}") for kp in range(NK // 2)]
        wo_s = [wo_pool.tile([128, HID], BF16, tag=f"wo{k}",
                             name=f"wo_{k}") for k in range(NK)]

        def load_rest():
            for kp in range(NK // 2):
                nc.gpsimd.dma_start(
                    wg8_s[kp][:, 0:HID],
                    wg8.ap()[256 * kp:256 * kp + 128, :])
                nc.gpsimd.dma_start(
                    wg8_s[kp][:, HID:2 * HID],
                    wg8.ap()[256 * kp + 128:256 * kp + 256, :])
            for k in range(NK):
                nc.gpsimd.dma_start(
                    wo_s[k][:], woutT.ap()[128 * k:128 * (k + 1), :])

        # ---- per-iteration emission helpers ------------------------------
        def emit_x(i):
            """x tiles for iteration i (1..NG): cols [HALO+(i-1)*WG, ...)."""
            tok = slice(HALO + (i - 1) * WG, HALO + i * WG)
            xts = []
            for k in range(NK):
                t = xt_pool.tile([128, WG], BF16, tag="xt", name=f"xt_{i}_{k}")
                nc.sync.dma_start(t[:], xT.ap()[128 * k:128 * (k + 1), tok])
                xts.append(t)
            return xts

        def phi_drain(ps, dst, w, name):
            """dst[:, :w] = 32*phi(ps/32) = max(ps+32, 32*min(exp(ps/32),1))"""
            e = et_pool.tile([128, WG], BF16, tag="e", name=f"e_{name}")
            nc.scalar.activation(e[:, 0:w], ps[:, 0:w], AF.Exp, scale=1.0 / S)
            tm = et_pool.tile([128, WG], BF16, tag="tm", name=f"tm_{name}")
            nc.vector.tensor_scalar(tm[:, 0:w], e[:, 0:w], 1.0, S,
                                    AL.min, AL.mult)
            nc.vector.scalar_tensor_tensor(dst[:, 0:w], ps[:, 0:w], S,
                                           tm[:, 0:w], AL.add, AL.max)

        def emit_sec(i, sec, xts, drain, w=WG, xoff=0):
            """One qkv section: 8 psum tiles, each drained via drain(j, ps)."""
            for j in range(NH):
                oti = j
                ps = ps_pool.tile([128, WG], F32, tag="mm",
                                  name=f"ps_{i}_{sec}_{j}")
                for k in range(NK):
                    nc.tensor.matmul(
                        ps[:, 0:w],
                        wq_sec[sec][k][:, 128 * oti:128 * (oti + 1)],
                        xts[k][:, xoff:xoff + w],
                        start=(k == 0), stop=(k == NK - 1))
                drain(j, ps)

        state = {"ks": [None] * NH, "kv": [None] * NH}

        def emit_scans(i, which, data, out_w):
            """8 scans of width out_w; returns cum tiles [128, W1]."""
            cums = []
            for j in range(NH):
                dec_b = dec_s[:, j:j + 1].broadcast_to([128, out_w])
                cum = cum_pool.tile([128, W1], BF16, tag=f"c{which}{j}",
                                    name=f"c{which}_{i}_{j}")
                init = 0.0 if i == 1 else state[which][j][:, 0:1]
                nc.vector.tensor_tensor_scan(
                    cum[:, 0:out_w], dec_b, data[j][:, 0:out_w], init,
                    AL.mult, AL.add)
                cums.append(cum)
            return cums

        def emit_state(i, which, cums, out_w):
            if i == NG:
                return
            nxt = []
            for j in range(NH):
                s = st_pool.tile([128, 1], F32, tag=f"s{which}{j}",
                                 name=f"s{which}_{i}_{j}")
                nc.gpsimd.tensor_copy(s[:], cums[j][:, out_w - 1:out_w])
                nxt.append(s)
            state[which] = nxt

        def emit_gate_mix(p_oa8, p_dls, p_v1, p_i):
            """fp8 DoubleRow gate matmul, sigmoid, and mix = g*(oa-v)+v,
            interleaved per output tile to keep the gts live set at 2."""
            mixes = []
            for ot in range(NH):
                ps = ps_pool.tile([128, WG], F32, tag="mm",
                                  name=f"gp_{p_i}_{ot}")
                for kp in range(NK // 2):
                    lhs = wg8_s[kp][:, :].rearrange(
                        "p (i m) -> p i m", i=2)[:, :, 128 * ot:128 * (ot + 1)]
                    rhs = p_oa8[kp][:, :].rearrange("p (i n) -> p i n", i=2)
                    nc.tensor.matmul(ps[:], lhs, rhs, start=(kp == 0),
                                     stop=(kp == NK // 2 - 1), perf_mode=DR)
                g = gt_pool.tile([128, WG], BF16, tag="gt",
                                 name=f"gt_{p_i}_{ot}")
                nc.scalar.activation(g[:], ps[:], AF.Sigmoid,
                                     bias=bgate_s[:, ot:ot + 1], scale=GS)
                m1 = et_pool.tile([128, WG], BF16, tag="m1",
                                  name=f"m1_{p_i}_{ot}")
                nc.vector.tensor_tensor(m1[:], g[:], p_dls[ot][:], AL.mult)
                mx = mx_pool.tile([128, WG], BF16, tag="mx",
                                  name=f"mx_{p_i}_{ot}")
                nc.vector.tensor_tensor(mx[:], m1[:], p_v1[ot][:], AL.add)
                mixes.append(mx)
            return mixes

        def emit_out(mixes, p_i):
            out_tok = slice((p_i - 1) * WG, p_i * WG)
            for ot in range(NH):
                ps = ps_pool.tile([128, WG], F32, tag="mm",
                                  name=f"yp_{p_i}_{ot}")
                for k in range(NK):
                    nc.tensor.matmul(
                        ps[:], wo_s[k][:, 128 * ot:128 * (ot + 1)],
                        mixes[k][:], start=(k == 0), stop=(k == NK - 1))
                ysb = y_pool.tile([128, WG], F32, tag="ysb",
                                  name=f"ysb_{p_i}_{ot}")
                nc.scalar.activation(ysb[:], ps[:], AF.Copy, scale=1.0 / S)
                nc.sync.dma_start(yT.ap()[128 * ot:128 * (ot + 1), out_tok],
                                  ysb[:])

        # ================= prologue: halo ================================
        xh = []
        for k in range(NK):
            t = xt_pool.tile([128, HALO], BF16, tag="xh", bufs=8,
                             name=f"xh_{k}")
            nc.sync.dma_start(t[:], xT.ap()[128 * k:128 * (k + 1), 0:HALO])
            xh.append(t)
        x1 = emit_x(1)
        load_wq_sec(2)  # v-section
        load_wq_sec(0)  # q-section

        k1_1 = [k1_pool.tile([128, W1], BF16, tag="k1", name=f"k1_1_{j}")
                for j in range(NH)]
        kvs_1 = [kv_pool.tile([128, W1], BF16, tag=f"kv{j}", name=f"kvs_1_{j}")
                 for j in range(NH)]

        for j in range(NH):  # halo k-section
            ps = ps_pool.tile([128, WG], F32, tag="mm", name=f"psh_k_{j}")
            for k in range(NK):
                nc.tensor.matmul(ps[:, 0:HALO],
                                 wq_sec[1][k][:, 128 * j:128 * (j + 1)],
                                 xh[k][:, 0:HALO],
                                 start=(k == 0), stop=(k == NK - 1))
            kr = et_pool.tile([128, HALO], BF16, tag="kr", name=f"krh_{j}")
            phi_drain(ps, kr, HALO, f"kh_{j}")
            # mask: half-0 cores zero the halo k so scan state starts at 0
            nc.vector.tensor_scalar_mul(k1_1[j][:, 0:HALO], kr[:, 0:HALO],
                                        mask_s[:, 0:1])
        for j in range(NH):  # halo v-section; k*v goes straight to kvs_1
            ps = ps_pool.tile([128, WG], F32, tag="mm", name=f"psh_v_{j}")
            for k in range(NK):
                nc.tensor.matmul(ps[:, 0:HALO],
                                 wq_sec[2][k][:, 128 * j:128 * (j + 1)],
                                 xh[k][:, 0:HALO],
                                 start=(k == 0), stop=(k == NK - 1))
            vh = et_pool.tile([128, HALO], BF16, tag="vh", name=f"vh_{j}")
            nc.scalar.copy(vh[:], ps[:, 0:HALO])
            nc.gpsimd.tensor_tensor(kvs_1[j][:, 0:HALO], k1_1[j][:, 0:HALO],
                                    vh[:], AL.mult)
        load_rest()

        # ================= main loop =====================================
        prev = None  # (oa8_pairs, dls, v1, i)
        xs = {1: x1}
        for i in range(1, NG + 1):
            koff = HALO if i == 1 else 0
            w1 = W1 if i == 1 else WG
            xts = xs.pop(i)
            if i < NG:
                xs[i + 1] = emit_x(i + 1)

            if i == 1:
                k1_i, kvs_i = k1_1, kvs_1
            else:
                k1_i = [k1_pool.tile([128, W1], BF16, tag="k1",
                                     name=f"k1_{i}_{j}") for j in range(NH)]
                kvs_i = [kv_pool.tile([128, W1], BF16, tag=f"kv{j}",
                                      name=f"kvs_{i}_{j}") for j in range(NH)]
            v1_i = [v1_pool.tile([128, WG], BF16, tag="v1",
                                 name=f"v1_{i}_{j}") for j in range(NH)]

            # PE: k-section; DVE/ACT: phi-k drains
            def drain_k(j, ps, k1_i=k1_i, koff=koff, i=i):
                # write into [koff : koff+WG]
                kview = k1_i[j][:, koff:koff + WG]
                e = et_pool.tile([128, WG], BF16, tag="e", name=f"e_k{i}_{j}")
                nc.scalar.activation(e[:], ps[:], AF.Exp, scale=1.0 / S)
                tm = et_pool.tile([128, WG], BF16, tag="tm",
                                  name=f"tm_k{i}_{j}")
                nc.vector.tensor_scalar(tm[:], e[:], 1.0, S, AL.min, AL.mult)
                nc.vector.scalar_tensor_tensor(kview, ps[:], S, tm[:],
                                               AL.add, AL.max)
            emit_sec(i, 1, xts, drain_k)

            # PE: q-section; phi-q drains
            q1_i = [q1_pool.tile([128, WG], BF16, tag="q1",
                                 name=f"q1_{i}_{j}") for j in range(NH)]

            def drain_q(j, ps, q1_i=q1_i, i=i):
                e = et_pool.tile([128, WG], BF16, tag="e", name=f"e_q{i}_{j}")
                nc.scalar.activation(e[:], ps[:], AF.Exp, scale=1.0 / S)
                tm = et_pool.tile([128, WG], BF16, tag="tm",
                                  name=f"tm_q{i}_{j}")
                nc.vector.tensor_scalar(tm[:], e[:], 1.0, S, AL.min, AL.mult)
                nc.vector.scalar_tensor_tensor(q1_i[j][:], ps[:], S, tm[:],
                                               AL.add, AL.max)
            emit_sec(i, 0, xts, drain_q)

            # DVE: ksum scans + prods
            cks = emit_scans(i, "ks", k1_i, w1)
            prods = []
            for j in range(NH):
                pr = pr_pool.tile([128, WG], BF16, tag="pr",
                                  name=f"pr_{i}_{j}")
                nc.vector.tensor_tensor(pr[:], q1_i[j][:],
                                        cks[j][:, koff:koff + WG], AL.mult)
                prods.append(pr)
            emit_state(i, "ks", cks, w1)

            # PE: gate matmul for prev iter (fp8 DoubleRow); sigmoid + mix
            if prev is not None:
                p_oa8, p_dls, p_v1, p_i = prev
                mixes = emit_gate_mix(p_oa8, p_dls, p_v1, p_i)

            # PE: v-section; ACT copies
            def drain_v(j, ps, v1_i=v1_i):
                nc.scalar.copy(v1_i[j][:], ps[:])
            emit_sec(i, 2, xts, drain_v)

            # gpsimd: k*v products
            for j in range(NH):
                nc.gpsimd.tensor_tensor(kvs_i[j][:, koff:koff + WG],
                                        k1_i[j][:, koff:koff + WG],
                                        v1_i[j][:], AL.mult)

            # PE: den selector matmul; DVE: clip (in-place), recip, cast
            dps = psd_pool.tile([H, WG], F32, tag="den", name=f"dps_{i}")
            for j in range(NH):
                nc.tensor.matmul(dps[:], densel_s[:, H * j:H * (j + 1)],
                                 prods[j][:], start=(j == 0),
                                 stop=(j == NH - 1))
            nc.vector.tensor_scalar_max(dps[:], dps[:], 1e-6 * S * S)
            den_f = den_pool.tile([H, WG], F32, tag="denf", name=f"denf_{i}")
            nc.vector.reciprocal_approx_fast(den_f[:], dps[:])
            den_i = den_pool.tile([H, WG], BF16, tag="deni", name=f"deni_{i}")
            nc.vector.tensor_scalar_mul(den_i[:], den_f[:], 1.0)

            # PE: out matmul for prev iter; ACT y copies; DMA out
            if prev is not None:
                emit_out(mixes, p_i)

            # Interleaved per j: kv scan (DVE), bc broadcast matmul (PE),
            # qckv/oa (DVE), oa8 cast (ACT), dls (gpsimd).  Interleaving
            # keeps at most ~2 bc psum banks live so the next iteration's
            # k-section is never starved of PSUM.
            oa8 = [oa8_pool.tile([128, 2 * WG], FP8, tag=f"o8{kp}",
                                 name=f"oa8_{i}_{kp}")
                   for kp in range(NH // 2)]
            dls = []
            ckv = emit_scans(i, "kv", kvs_i, w1)
            for j in range(NH):
                cum = ckv[j]
                bc = ps_pool.tile([128, WG], F32, tag="mm",
                                  name=f"bc_{i}_{j}")
                nc.tensor.matmul(bc[:], bcsel_s[:, 128 * j:128 * (j + 1)],
                                 den_i[:, :], start=True, stop=True)
                qc = et_pool.tile([128, WG], BF16, tag="qc", bufs=2,
                                  name=f"qc_{i}_{j}")
                nc.vector.tensor_tensor(qc[:], q1_i[j][:],
                                        cum[:, koff:koff + WG], AL.mult)
                oa = oa_pool.tile([128, WG], BF16, tag="oa",
                                  name=f"oa_{i}_{j}")
                nc.vector.tensor_tensor(oa[:], qc[:], bc[:], AL.mult)
                nc.scalar.activation(
                    oa8[j // 2][:, WG * (j % 2):WG * (j % 2 + 1)],
                    oa[:], AF.Copy, scale=OSC)
                dl = dl_pool.tile([128, WG], BF16, tag="dl",
                                  name=f"dl_{i}_{j}")
                nc.gpsimd.tensor_tensor(dl[:], oa[:], v1_i[j][:], AL.subtract)
                dls.append(dl)
            emit_state(i, "kv", ckv, w1)

            prev = (oa8, dls, v1_i, i)

        # ================= epilogue: gate/mix/out for last iter ==========
        p_oa8, p_dls, p_v1, p_i = prev
        mixes = emit_gate_mix(p_oa8, p_dls, p_v1, p_i)
        emit_out(mixes, p_i)

    nc.compile()
    return nc


def _sigmoid(v):
    return 1.0 / (1.0 + np.exp(-v))


def _make_inputs(x, Wqkv, Wout, Wgate, bgate, decay_param):
    decay = _sigmoid(np.asarray(decay_param, np.float64)).astype(np.float32)
    bf = ml_dtypes.bfloat16
    f8 = ml_dtypes.float8_e4m3
    # x32: the whole pipeline rides this scale (see module docstring);
    # scaling by a power of two is exact in bf16.
    wqkvT = np.ascontiguousarray(
        np.asarray(Wqkv, np.float32).T * np.float32(S)).astype(bf)
    wg8 = np.ascontiguousarray(
        np.asarray(Wgate, np.float32).T * np.float32(S)).astype(f8)
    woutT = np.ascontiguousarray(np.asarray(Wout, np.float32).T).astype(bf)

    p = np.arange(128)
    dec_c = np.empty((128, NH), np.float32)
    for j in range(NH):
        dec_c[:, j] = decay[2 * j + p // 64]
    densel = np.zeros((128, NH * H), np.float32)
    for j in range(NH):
        for pp in range(128):
            densel[pp, H * j + 2 * j + pp // 64] = 1.0
    bcsel = np.zeros((H, NH * 128), np.float32)
    for j in range(NH):
        for m in range(128):
            bcsel[2 * j + m // 64, 128 * j + m] = 1.0
    bgate_c = np.ascontiguousarray(
        np.asarray(bgate, np.float32).reshape(NH, 128).T)

    in_maps = []
    for c in range(8):
        b, half = c // 2, c % 2
        xb = np.asarray(x[b], np.float32)  # [T, HID]
        if half == 0:
            xloc = np.concatenate(
                [np.zeros((HALO, HID), np.float32), xb[:HALF_T]], axis=0)
            mask = np.zeros((128, 1), np.float32)
        else:
            xloc = xb[HALF_T - HALO:]
            mask = np.ones((128, 1), np.float32)
        in_maps.append({
            "xT": np.ascontiguousarray(xloc.T).astype(bf),
            "wqkvT": wqkvT, "wg8": wg8, "woutT": woutT,
            "dec_c": dec_c, "mask_c": mask,
            "densel": densel.astype(bf), "bcsel": bcsel.astype(bf),
            "bgate_c": bgate_c,
        })
    return in_maps


def kernel(x, Wqkv, Wout, Wgate, bgate, decay_param):
    if "nc" not in _cache:
        _cache["nc"] = _build_nc()
    nc = _cache["nc"]
    in_maps = _make_inputs(x, Wqkv, Wout, Wgate, bgate, decay_param)
    res = run_bass_kernel_spmd(nc, in_maps, list(range(8)))
    y = np.empty((B, T, HID), np.float32)
    for c in range(8):
        b, half = c // 2, c % 2
        y[b, half * HALF_T:(half + 1) * HALF_T, :] = res.results[c]["yT"].T
    return y
